# revision 1
# baseline (speedup 1.0000x reference)
"""Trainium2 Bass kernel for nn_Conv2D_ConvLSTM1D (Conv2D stack -> 2x ConvLSTM1D -> FC).

Sharding: data-parallel over batch. 64 batches / 8 cores = 8 per core.
Each core runs the identical program on its batch shard; no collectives.

Per-core layout: channels on SBUF partitions, (batch, station) on the free
dim.  All convolutions become PE matmuls: contract dim = input channels
(or 3x3 taps for conv1), shifts over stations/time handled by sliding AP
windows into zero-padded SBUF tiles.  The whole network runs per-time-step
so every intermediate stays SBUF-resident (no DRAM spill):

  t-loop:  conv1(t+1) -> conv2(t) -> L2norm -> [LSTM1 gates/cell]
           -> [LSTM2 gates/cell]   (LSTM x-convs accumulate into the same
           PSUM tiles as the recurrent convs)

Matmul operands are bf16 (fp32 PSUM accumulation, fp32 cell state and gate
math) - measured end-to-end relative error vs the fp32 reference ~4e-3.
"""
import sys

if "/opt/trn_rl_repo" not in sys.path:
    sys.path.insert(0, "/opt/trn_rl_repo")

import numpy as np

B_FULL, T, N, CIN = 64, 24, 256, 1
F1, F2 = 64, 128
NCORES = 8
BL = B_FULL // NCORES          # batch per core
PAIRS = BL // 2                # matmuls process 2 batches (512 cols) at once

_CACHE = {}


def _legalize_waits(nc, mybir):
    """TRN2 engine instructions carry at most 1 sync wait (EventSemaphore: 2).
    Tile emits more; split extras onto injected NoOps placed just before."""
    n = 0
    for _, bbobj in list(nc.bb_map.items()):
        bb = bbobj.bb if hasattr(bbobj, "bb") else bbobj
        insts = bb.instructions
        newlist = []
        for inst in insts:
            si = getattr(inst, "sync_info", None)
            waits = list(si.on_wait) if (si is not None and si.on_wait) else []
            cap = 2 if isinstance(inst, mybir.InstEventSemaphore) else 1
            if len(waits) > cap:
                extra, keep = waits[:-cap], waits[-cap:]
                for w in extra:
                    n += 1
                    nop = mybir.InstNoOp(name=f"I-wf{n}", ins=[], outs=[])
                    nop.engine = inst.engine
                    nop.sync_info = mybir.SyncInfo(on_wait=[w], on_update=[])
                    nc.inst_map[nop.name] = nop
                    newlist.append(nop)
                inst.sync_info = mybir.SyncInfo(
                    on_wait=keep,
                    on_update=list(si.on_update) if si.on_update else [])
            newlist.append(inst)
        insts[:] = newlist
    return n


def _build_nc():
    import concourse.bass as bass
    import concourse.tile as tile
    from concourse import mybir
    from concourse.alu_op_type import AluOpType as Op

    F32 = mybir.dt.float32
    BF16 = mybir.dt.bfloat16
    AF = mybir.ActivationFunctionType
    AX = mybir.AxisListType

    nc = bass.Bass()

    x_d = nc.dram_tensor("x", [BL, T, N, CIN], F32, kind="ExternalInput")
    c1w_d = nc.dram_tensor("conv1_w", [3, 3, CIN, F1], F32, kind="ExternalInput")
    c1b_d = nc.dram_tensor("conv1_b", [F1], F32, kind="ExternalInput")
    c2w_d = nc.dram_tensor("conv2_w", [3, 3, F1, F1], F32, kind="ExternalInput")
    c2b_d = nc.dram_tensor("conv2_b", [F1], F32, kind="ExternalInput")
    kx1_d = nc.dram_tensor("kx1", [3, F1, 4 * F1], F32, kind="ExternalInput")
    kh1_d = nc.dram_tensor("kh1", [3, F1, 4 * F1], F32, kind="ExternalInput")
    b1_d = nc.dram_tensor("b1", [4 * F1], F32, kind="ExternalInput")
    kx2_d = nc.dram_tensor("kx2", [3, F1, 4 * F2], F32, kind="ExternalInput")
    kh2_d = nc.dram_tensor("kh2", [3, F2, 4 * F2], F32, kind="ExternalInput")
    b2_d = nc.dram_tensor("b2", [4 * F2], F32, kind="ExternalInput")
    fcw_d = nc.dram_tensor("fc_w", [F2, 1], F32, kind="ExternalInput")
    fcb_d = nc.dram_tensor("fc_b", [1], F32, kind="ExternalInput")
    y_d = nc.dram_tensor("y", [BL, N], F32, kind="ExternalOutput")

    xd = x_d.rearrange("b t n c -> b t (n c)")  # [BL, T, N]

    with tile.TileContext(nc) as tc:
        with (
            tc.tile_pool(name="wp", bufs=1) as wp,
            tc.tile_pool(name="st", bufs=1) as st,
            tc.tile_pool(name="io", bufs=2) as io,
            tc.tile_pool(name="gt", bufs=1) as gt,
            tc.tile_pool(name="sc", bufs=3) as scp,
            tc.tile_pool(name="pp", bufs=6, space="PSUM") as pp,
            tc.tile_pool(name="pS", bufs=2, space="PSUM") as pS,
        ):
            # ---------------- setup: weights (bf16 via casting gpsimd DMA) ----
            w1 = wp.tile([9, F1], BF16)
            nc.gpsimd.dma_start(w1[:], c1w_d.rearrange("a b c d -> (a b c) d"))
            w2 = wp.tile([F1, 9, F1], BF16)
            nc.gpsimd.dma_start(w2[:], c2w_d.rearrange("a b c d -> c (a b) d"))
            kx1 = wp.tile([F1, 3, 4 * F1], BF16)
            nc.gpsimd.dma_start(kx1[:], kx1_d.rearrange("k c f -> c k f"))
            kh1 = wp.tile([F1, 3, 4 * F1], BF16)
            nc.gpsimd.dma_start(kh1[:], kh1_d.rearrange("k c f -> c k f"))
            kx2 = wp.tile([F1, 3, 4 * F2], BF16)
            nc.gpsimd.dma_start(kx2[:], kx2_d.rearrange("k c f -> c k f"))
            kh2 = wp.tile([F2, 3, 4 * F2], BF16)
            nc.gpsimd.dma_start(kh2[:], kh2_d.rearrange("k c f -> c k f"))
            fcw = wp.tile([F2, 1], BF16)
            nc.gpsimd.dma_start(fcw[:], fcw_d[:])

            # biases / consts (fp32)
            c1b = wp.tile([F1, 1], F32)
            nc.sync.dma_start(c1b[:], c1b_d.rearrange("(f u) -> f u", u=1))
            c2b = wp.tile([F1, 1], F32)
            nc.sync.dma_start(c2b[:], c2b_d.rearrange("(f u) -> f u", u=1))
            b1s = wp.tile([128, 2], F32)
            nc.sync.dma_start(b1s[:], b1_d.rearrange("(g p) -> p g", g=2))
            b2s = wp.tile([128, 4], F32)
            nc.sync.dma_start(b2s[:], b2_d.rearrange("(g p) -> p g", g=4))
            fcb = wp.tile([1, 1], F32)
            nc.sync.dma_start(fcb[:], fcb_d.rearrange("(f u) -> f u", u=1))

            # hard-sigmoid-folded additive consts:
            #   i/f/o gates: gate = clip(0.2*psum + (0.2*b + 0.5), 0, 1)
            #   gc gate:     gate = tanh(psum + b)
            addc1 = wp.tile([128, 2], F32)
            nc.vector.tensor_scalar(out=addc1[:, 0:1], in0=b1s[:, 0:1],
                                    scalar1=0.2, scalar2=0.5,
                                    op0=Op.mult, op1=Op.add)
            nc.vector.tensor_copy(addc1[0:64, 1:2], b1s[0:64, 1:2])  # gc rows
            nc.vector.tensor_scalar(out=addc1[64:128, 1:2], in0=b1s[64:128, 1:2],
                                    scalar1=0.2, scalar2=0.5,
                                    op0=Op.mult, op1=Op.add)          # o rows
            addc2 = wp.tile([128, 4], F32)
            nc.vector.tensor_scalar(out=addc2[:], in0=b2s[:],
                                    scalar1=0.2, scalar2=0.5,
                                    op0=Op.mult, op1=Op.add)
            nc.vector.tensor_copy(addc2[:, 2:3], b2s[:, 2:3])         # gc group
            eps = wp.tile([128, 1], F32)
            nc.vector.memset(eps[:], 1e-12)
            zero = wp.tile([128, 1], F32)
            nc.vector.memset(zero[:], 0.0)
            ones = wp.tile([128, 128], F32)
            nc.vector.memset(ones[:], 1.0)

            # ---------------- persistent state ----------------
            hconv = st.tile([F1, 3, BL, N + 2], BF16)   # conv1 out, 3-slot t-window
            nc.vector.memset(hconv[:], 0.0)
            h1p = st.tile([F1, BL, N + 2], BF16)        # LSTM1 hidden (padded)
            nc.vector.memset(h1p[:], 0.0)
            c1t = st.tile([F1, BL, N], F32)             # LSTM1 cell
            nc.vector.memset(c1t[:], 0.0)
            h2p = st.tile([F2, BL, N + 2], BF16)        # LSTM2 hidden (padded)
            nc.vector.memset(h2p[:], 0.0)
            c2t = st.tile([F2, BL, N], F32)             # LSTM2 cell
            nc.vector.memset(c2t[:], 0.0)

            # double-buffered im2col / hn with persistent zero edges
            imt = [st.tile([9, BL, N], BF16, tag=f"im{i}", name=f"im{i}") for i in range(2)]
            for tl in imt:
                nc.vector.memset(tl[:], 0.0)
            hnt = [st.tile([F1, BL, N + 2], BF16, tag=f"hn{i}", name=f"hn{i}") for i in range(2)]
            for tl in hnt:
                nc.vector.memset(tl[:], 0.0)

            # gate tiles (full batch width)
            G0 = gt.tile([128, BL, N], F32, tag="G0")   # L1: i|f
            G1 = gt.tile([128, BL, N], F32, tag="G1")   # L1: gc|o
            G2 = [gt.tile([128, BL, N], F32, tag=f"L2g{g}", name=f"L2g{g}") for g in range(4)]

            def pr(a, p):  # batch-pair slice helper on dim after partitions
                return a[:, 2 * p:2 * p + 2, :]

            # ------------- per-time-step pieces -------------
            def conv1_time(tt):
                """im2col DMA + matmul; writes hconv slot tt%3 (bias folded)."""
                im = imt[tt % 2]
                if tt == 0 or tt == T - 1:
                    # t-edge: some taps fall outside [0,T); clear whole tile
                    # (partition-base of row-range memsets must be 32-aligned)
                    nc.vector.memset(im[:], 0.0)
                for dt in range(3):
                    ts = tt + dt - 1
                    if not (0 <= ts < T):
                        continue
                    for dn in range(3):
                        r = 3 * dt + dn
                        if dn == 0:
                            nc.gpsimd.dma_start(im[r:r + 1, :, 1:N],
                                                xd[:, ts, 0:N - 1])
                        elif dn == 1:
                            nc.gpsimd.dma_start(im[r:r + 1, :, :],
                                                xd[:, ts, :])
                        else:
                            nc.gpsimd.dma_start(im[r:r + 1, :, 0:N - 1],
                                                xd[:, ts, 1:N])
                s = tt % 3
                for p in range(PAIRS):
                    ps = pp.tile([128, 2, N], F32, tag="pair")
                    nc.tensor.matmul(ps[0:F1], w1[:], pr(im, p),
                                     start=True, stop=True)
                    nc.scalar.activation(hconv[:, s, 2 * p:2 * p + 2, 1:N + 1],
                                         ps[0:F1], AF.Identity,
                                         bias=c1b[:], scale=1.0)

            def conv2_l2_hn(t):
                """conv2 + bias, L2-normalize over (N,C) per (b,t), write hn."""
                g2v = io.tile([F1, BL, N], F32, tag="g2")
                for p in range(PAIRS):
                    ps = pp.tile([128, 2, N], F32, tag="pair")
                    for i9 in range(9):
                        dt, dn = divmod(i9, 3)
                        s = (t + dt - 1) % 3
                        nc.tensor.matmul(
                            ps[0:F1], w2[:, i9, :],
                            hconv[:, s, 2 * p:2 * p + 2, dn:dn + N],
                            start=(i9 == 0), stop=(i9 == 8))
                    nc.scalar.activation(pr(g2v, p), ps[0:F1], AF.Identity,
                                         bias=c2b[:], scale=1.0)
                sq = io.tile([F1, BL, N], F32, tag="sq")
                nc.vector.tensor_mul(sq[:], g2v[:], g2v[:])
                ns = scp.tile([F1, BL], F32, tag="ns")
                nc.vector.tensor_reduce(ns[:], sq[:], AX.X, Op.add)
                psS = pS.tile([128, BL], F32, tag="S")
                nc.tensor.matmul(psS[:], ones[0:F1, :], ns[:],
                                 start=True, stop=True)
                srt = scp.tile([128, BL], F32, tag="srt")
                nc.scalar.activation(srt[:], psS[:], AF.Sqrt,
                                     bias=eps[:], scale=1.0)
                scl = scp.tile([128, BL], F32, tag="scl")
                nc.vector.reciprocal(scl[:], srt[:])
                hn = hnt[t % 2]
                nc.vector.tensor_mul(hn[:, :, 1:N + 1], g2v[:],
                                     scl[0:F1, :].to_broadcast((F1, BL, N)))
                return hn

            def lstm_step(t, F, G, kx, kh, xin, hp, ct, gtiles, addc, gc_g):
                """One ConvLSTM1D step. gtiles: list of gate tiles, one per
                128-partition group. gc_g: group index holding tanh-gate."""
                ngrp = 4 * F // 128
                for g in range(ngrp):
                    gs = slice(128 * g, 128 * (g + 1))
                    for p in range(PAIRS):
                        ps = pp.tile([128, 2, N], F32, tag="pair")
                        for k in range(3):
                            nc.tensor.matmul(ps[:], kh[:, k, gs],
                                             hp[:, 2 * p:2 * p + 2, k:k + N],
                                             start=(k == 0), stop=False)
                        for k in range(3):
                            nc.tensor.matmul(ps[:], kx[:, k, gs],
                                             xin[:, 2 * p:2 * p + 2, k:k + N],
                                             start=False, stop=(k == 2))
                        gv = gtiles[g]
                        if F == 128:
                            if g == gc_g:
                                nc.scalar.activation(pr(gv, p), ps[:], AF.Tanh,
                                                     bias=addc[:, g:g + 1],
                                                     scale=1.0)
                            else:
                                nc.vector.tensor_scalar(
                                    out=pr(gv, p), in0=ps[:],
                                    scalar1=0.2, scalar2=addc[:, g:g + 1],
                                    op0=Op.mult, op1=Op.add)
                                nc.gpsimd.tensor_scalar(
                                    out=pr(gv, p), in0=pr(gv, p),
                                    scalar1=0.0, scalar2=1.0,
                                    op0=Op.max, op1=Op.min)
                        else:
                            # F=64: g0 = i|f (both hard-sigmoid),
                            #       g1 = gc|o (tanh rows 0:64, hs rows 64:128)
                            if g == 0:
                                nc.vector.tensor_scalar(
                                    out=pr(gv, p), in0=ps[:],
                                    scalar1=0.2, scalar2=addc[:, 0:1],
                                    op0=Op.mult, op1=Op.add)
                                nc.gpsimd.tensor_scalar(
                                    out=pr(gv, p), in0=pr(gv, p),
                                    scalar1=0.0, scalar2=1.0,
                                    op0=Op.max, op1=Op.min)
                            else:
                                nc.scalar.activation(
                                    pr(gv, p)[0:64], ps[0:64], AF.Tanh,
                                    bias=addc[0:64, 1:2], scale=1.0)
                                nc.vector.tensor_scalar(
                                    out=pr(gv, p)[64:128], in0=ps[64:128],
                                    scalar1=0.2, scalar2=addc[64:128, 1:2],
                                    op0=Op.mult, op1=Op.add)
                                nc.gpsimd.tensor_scalar(
                                    out=pr(gv, p)[64:128], in0=pr(gv, p)[64:128],
                                    scalar1=0.0, scalar2=1.0,
                                    op0=Op.max, op1=Op.min)
                # cell update per pair
                for p in range(PAIRS):
                    if F == 128:
                        iv, fv = pr(gtiles[0], p), pr(gtiles[1], p)
                        gcv, ov = pr(gtiles[2], p), pr(gtiles[3], p)
                    else:
                        iv = pr(gtiles[0], p)[0:64]
                        gcv = pr(gtiles[1], p)[0:64]
                        # f and o live on partitions 64:128; DMA-copy down
                        fv = scp.tile([64, 2, N], F32, tag="fc")
                        nc.sync.dma_start(fv[:], pr(gtiles[0], p)[64:128])
                        ov = scp.tile([64, 2, N], F32, tag="oc")
                        nc.sync.dma_start(ov[:], pr(gtiles[1], p)[64:128])
                    m1 = scp.tile([F, 2, N], F32, tag="m1")
                    nc.vector.tensor_mul(m1[:], fv[:] if F == 64 else fv, pr(ct, p))
                    m2 = scp.tile([F, 2, N], F32, tag="m2")
                    nc.vector.tensor_mul(m2[:], iv, gcv)
                    nc.vector.tensor_add(pr(ct, p), m1[:], m2[:])
                    tcv = scp.tile([F, 2, N], F32, tag="tc")
                    nc.scalar.activation(tcv[:], pr(ct, p), AF.Tanh,
                                         bias=zero[0:F, :], scale=1.0)
                    nc.vector.tensor_mul(hp[:, 2 * p:2 * p + 2, 1:N + 1],
                                         ov[:] if F == 64 else ov, tcv[:])

            # ---------------- main time loop ----------------
            conv1_time(0)
            for t in range(T):
                if t + 1 < T:
                    conv1_time(t + 1)
                else:
                    nc.vector.memset(hconv[:, (t + 1) % 3, :, :], 0.0)
                hn = conv2_l2_hn(t)
                lstm_step(t, F1, 2, kx1, kh1, hn, h1p, c1t, [G0, G1], addc1, 1)
                lstm_step(t, F2, 4, kx2, kh2, h1p, h2p, c2t, G2, addc2, 2)

            # ---------------- final L2 norm + FC ----------------
            sq2 = io.tile([F2, BL, N], F32, tag="sq2")
            nc.vector.tensor_mul(sq2[:], h2p[:, :, 1:N + 1], h2p[:, :, 1:N + 1])
            ns2 = scp.tile([F2, BL], F32, tag="ns")
            nc.vector.tensor_reduce(ns2[:], sq2[:], AX.X, Op.add)
            psS2 = pS.tile([128, BL], F32, tag="S")
            nc.tensor.matmul(psS2[:], ones[:], ns2[:], start=True, stop=True)
            srt2 = scp.tile([128, BL], F32, tag="srt")
            nc.scalar.activation(srt2[:], psS2[:], AF.Sqrt, bias=eps[:], scale=1.0)
            scl2 = scp.tile([128, BL], F32, tag="scl")
            nc.vector.reciprocal(scl2[:], srt2[:])
            ysb = io.tile([1, BL, N], F32, tag="y")
            for p in range(PAIRS):
                psY = pp.tile([128, 2, N], F32, tag="pair")
                nc.tensor.matmul(psY[0:1], fcw[:], h2p[:, 2 * p:2 * p + 2, 1:N + 1],
                                 start=True, stop=True)
                nc.vector.tensor_mul(pr(ysb, p), psY[0:1],
                                     scl2[0:1, 2 * p:2 * p + 2]
                                     .to_broadcast((1, 2, N)))
                nc.vector.tensor_scalar_add(pr(ysb, p), pr(ysb, p),
                                            fcb[0:1, 0:1])
            nc.sync.dma_start(y_d.rearrange("(u b) n -> u b n", u=1), ysb[:])

    from concourse import mybir as _mybir
    _legalize_waits(nc, _mybir)
    return nc


def kernel(**inputs):
    from concourse.bass_utils import run_bass_kernel_spmd

    if "nc" not in _CACHE:
        _CACHE["nc"] = _build_nc()
    nc = _CACHE["nc"]

    x = np.ascontiguousarray(np.asarray(inputs["x"], dtype=np.float32))
    shared = {k: np.ascontiguousarray(np.asarray(v, dtype=np.float32))
              for k, v in inputs.items() if k != "x"}
    shared["fc_w"] = shared["fc_w"].reshape(F2, 1)
    in_maps = []
    for c in range(NCORES):
        m = dict(shared)
        m["x"] = x[c * BL:(c + 1) * BL]
        in_maps.append(m)

    res = run_bass_kernel_spmd(nc, in_maps, core_ids=list(range(NCORES)))
    y = np.concatenate([res.results[c]["y"] for c in range(NCORES)], axis=0)
    return y.reshape(B_FULL, 1, N, 1).astype(np.float32)



# revision 4
# speedup vs baseline: 1.8296x; 1.8296x over previous
"""Trainium2 Bass kernel for nn_Conv2D_ConvLSTM1D (Conv2D stack -> 2x ConvLSTM1D -> FC).

Sharding: data-parallel over batch. 64 batches / 8 cores = 8 per core.
Each core runs the identical program on its batch shard; no collectives.

Per-core layout: channels on SBUF partitions, (batch, station) on the free
dim.  All convolutions become PE matmuls: contract dim = input channels
(or 3x3 taps for conv1), shifts over stations/time handled by sliding AP
windows into zero-padded SBUF tiles.  The whole network runs per-time-step
so every intermediate stays SBUF-resident (no DRAM spill):

  t-loop:  conv1(t+1) -> conv2(t) -> L2norm -> [LSTM1 gates/cell]
           -> [LSTM2 gates/cell]   (LSTM x-convs accumulate into the same
           PSUM tiles as the recurrent convs)

Matmul operands are bf16 (fp32 PSUM accumulation, fp32 cell state and gate
math) - measured end-to-end relative error vs the fp32 reference ~4e-3.
"""
import sys

if "/opt/trn_rl_repo" not in sys.path:
    sys.path.insert(0, "/opt/trn_rl_repo")

import numpy as np

B_FULL, T, N, CIN = 64, 24, 256, 1
F1, F2 = 64, 128
NCORES = 8
BL = B_FULL // NCORES          # batch per core
PAIRS = BL // 2                # matmuls process 2 batches (512 cols) at once

_CACHE = {}


def _legalize_waits(nc, mybir):
    """TRN2 engine instructions carry at most 1 sync wait (EventSemaphore: 2).
    Tile emits more; split extras onto injected NoOps placed just before."""
    n = 0
    for _, bbobj in list(nc.bb_map.items()):
        bb = bbobj.bb if hasattr(bbobj, "bb") else bbobj
        insts = bb.instructions
        newlist = []
        for inst in insts:
            si = getattr(inst, "sync_info", None)
            waits = list(si.on_wait) if (si is not None and si.on_wait) else []
            cap = 2 if isinstance(inst, mybir.InstEventSemaphore) else 1
            if len(waits) > cap:
                extra, keep = waits[:-cap], waits[-cap:]
                for w in extra:
                    n += 1
                    nop = mybir.InstNoOp(name=f"I-wf{n}", ins=[], outs=[])
                    nop.engine = inst.engine
                    nop.sync_info = mybir.SyncInfo(on_wait=[w], on_update=[])
                    nc.inst_map[nop.name] = nop
                    newlist.append(nop)
                inst.sync_info = mybir.SyncInfo(
                    on_wait=keep,
                    on_update=list(si.on_update) if si.on_update else [])
            newlist.append(inst)
        insts[:] = newlist
    return n


def _build_nc():
    import concourse.bass as bass
    import concourse.tile as tile
    from concourse import mybir
    from concourse.alu_op_type import AluOpType as Op

    F32 = mybir.dt.float32
    BF16 = mybir.dt.bfloat16
    AF = mybir.ActivationFunctionType
    AX = mybir.AxisListType

    nc = bass.Bass()

    x_d = nc.dram_tensor("x", [BL, T, N, CIN], F32, kind="ExternalInput")
    c1w_d = nc.dram_tensor("conv1_w", [3, 3, CIN, F1], F32, kind="ExternalInput")
    c1b_d = nc.dram_tensor("conv1_b", [F1], F32, kind="ExternalInput")
    c2w_d = nc.dram_tensor("conv2_w", [3, 3, F1, F1], F32, kind="ExternalInput")
    c2b_d = nc.dram_tensor("conv2_b", [F1], F32, kind="ExternalInput")
    kx1_d = nc.dram_tensor("kx1", [3, F1, 4 * F1], F32, kind="ExternalInput")
    kh1_d = nc.dram_tensor("kh1", [3, F1, 4 * F1], F32, kind="ExternalInput")
    b1_d = nc.dram_tensor("b1", [4 * F1], F32, kind="ExternalInput")
    kx2_d = nc.dram_tensor("kx2", [3, F1, 4 * F2], F32, kind="ExternalInput")
    kh2_d = nc.dram_tensor("kh2", [3, F2, 4 * F2], F32, kind="ExternalInput")
    b2_d = nc.dram_tensor("b2", [4 * F2], F32, kind="ExternalInput")
    fcw_d = nc.dram_tensor("fc_w", [F2, 1], F32, kind="ExternalInput")
    fcb_d = nc.dram_tensor("fc_b", [1], F32, kind="ExternalInput")
    y_d = nc.dram_tensor("y", [BL, N], F32, kind="ExternalOutput")

    xd = x_d.rearrange("b t n c -> b t (n c)")  # [BL, T, N]

    with tile.TileContext(nc) as tc:
        with (
            tc.tile_pool(name="wp", bufs=1) as wp,
            tc.tile_pool(name="st", bufs=1) as st,
            tc.tile_pool(name="io", bufs=2) as io,
            tc.tile_pool(name="gt", bufs=1) as gt,
            tc.tile_pool(name="sc", bufs=2) as scp,
            tc.tile_pool(name="pp", bufs=6, space="PSUM") as pp,
            tc.tile_pool(name="pS", bufs=2, space="PSUM") as pS,
        ):
            # ---------------- setup: weights (bf16 via casting gpsimd DMA) ----
            w1 = wp.tile([9, F1], BF16)
            nc.gpsimd.dma_start(w1[:], c1w_d.rearrange("a b c d -> (a b c) d"))
            w2 = wp.tile([F1, 9, F1], BF16)
            nc.gpsimd.dma_start(w2[:], c2w_d.rearrange("a b c d -> c (a b) d"))
            kx1 = wp.tile([F1, 3, 4 * F1], BF16)
            nc.gpsimd.dma_start(kx1[:], kx1_d.rearrange("k c f -> c k f"))
            kh1 = wp.tile([F1, 3, 4 * F1], BF16)
            nc.gpsimd.dma_start(kh1[:], kh1_d.rearrange("k c f -> c k f"))
            kx2 = wp.tile([F1, 3, 4 * F2], BF16)
            nc.gpsimd.dma_start(kx2[:], kx2_d.rearrange("k c f -> c k f"))
            kh2 = wp.tile([F2, 3, 4 * F2], BF16)
            nc.gpsimd.dma_start(kh2[:], kh2_d.rearrange("k c f -> c k f"))
            fcw = wp.tile([F2, 1], BF16)
            nc.gpsimd.dma_start(fcw[:], fcw_d[:])

            # biases / consts (fp32)
            c1b = wp.tile([F1, 1], F32)
            nc.sync.dma_start(c1b[:], c1b_d.rearrange("(f u) -> f u", u=1))
            c2b = wp.tile([F1, 1], F32)
            nc.sync.dma_start(c2b[:], c2b_d.rearrange("(f u) -> f u", u=1))
            b1s = wp.tile([128, 2], F32)
            nc.sync.dma_start(b1s[:], b1_d.rearrange("(g p) -> p g", g=2))
            b2s = wp.tile([128, 4], F32)
            nc.sync.dma_start(b2s[:], b2_d.rearrange("(g p) -> p g", g=4))
            fcb = wp.tile([1, 1], F32)
            nc.sync.dma_start(fcb[:], fcb_d.rearrange("(f u) -> f u", u=1))

            # hard-sigmoid-folded additive consts:
            #   i/f/o gates: gate = clip(0.2*psum + (0.2*b + 0.5), 0, 1)
            #   gc gate:     gate = tanh(psum + b)
            addc1 = wp.tile([128, 2], F32)
            nc.vector.tensor_scalar(out=addc1[:, 0:1], in0=b1s[:, 0:1],
                                    scalar1=0.2, scalar2=0.5,
                                    op0=Op.mult, op1=Op.add)
            nc.vector.tensor_copy(addc1[0:64, 1:2], b1s[0:64, 1:2])  # gc rows
            nc.vector.tensor_scalar(out=addc1[64:128, 1:2], in0=b1s[64:128, 1:2],
                                    scalar1=0.2, scalar2=0.5,
                                    op0=Op.mult, op1=Op.add)          # o rows
            addc2 = wp.tile([128, 4], F32)
            nc.vector.tensor_scalar(out=addc2[:], in0=b2s[:],
                                    scalar1=0.2, scalar2=0.5,
                                    op0=Op.mult, op1=Op.add)
            nc.vector.tensor_copy(addc2[:, 2:3], b2s[:, 2:3])         # gc group
            eps = wp.tile([128, 1], F32)
            nc.vector.memset(eps[:], 1e-12)
            zero = wp.tile([128, 1], F32)
            nc.vector.memset(zero[:], 0.0)
            ones = wp.tile([128, 128], F32)
            nc.vector.memset(ones[:], 1.0)

            # ---------------- persistent state ----------------
            hconv = st.tile([F1, 3, BL, N + 2], BF16)   # conv1 out, 3-slot t-window
            nc.vector.memset(hconv[:], 0.0)
            h1p = st.tile([F1, BL, N + 2], BF16)        # LSTM1 hidden (padded)
            nc.vector.memset(h1p[:], 0.0)
            c1t = st.tile([F1, BL, N], F32)             # LSTM1 cell
            nc.vector.memset(c1t[:], 0.0)
            h2p = st.tile([F2, BL, N + 2], BF16)        # LSTM2 hidden (padded)
            nc.vector.memset(h2p[:], 0.0)
            c2t = st.tile([F2, BL, N], F32)             # LSTM2 cell
            nc.vector.memset(c2t[:], 0.0)

            # double-buffered im2col / hn with persistent zero edges
            imt = [st.tile([9, BL, N], BF16, tag=f"im{i}", name=f"im{i}") for i in range(2)]
            for tl in imt:
                nc.vector.memset(tl[:], 0.0)
            hnt = [st.tile([F1, BL, N + 2], BF16, tag=f"hn{i}", name=f"hn{i}") for i in range(2)]
            for tl in hnt:
                nc.vector.memset(tl[:], 0.0)

            # gate tiles (full batch width)
            G0 = gt.tile([128, BL, N], F32, tag="G0")   # L1: i|f
            G1 = gt.tile([128, BL, N], F32, tag="G1")   # L1: gc|o
            G2 = [gt.tile([128, BL, N], F32, tag=f"L2g{g}", name=f"L2g{g}") for g in range(4)]

            def pr(a, p):  # batch-pair slice helper on dim after partitions
                return a[:, 2 * p:2 * p + 2, :]

            # ------------- per-time-step pieces -------------
            def conv1_time(tt):
                """im2col DMA + matmul; writes hconv slot tt%3 (bias folded)."""
                im = imt[tt % 2]
                if tt == 0 or tt == T - 1:
                    # t-edge: some taps fall outside [0,T); clear whole tile
                    # (partition-base of row-range memsets must be 32-aligned)
                    nc.vector.memset(im[:], 0.0)
                for dt in range(3):
                    ts = tt + dt - 1
                    if not (0 <= ts < T):
                        continue
                    for dn in range(3):
                        r = 3 * dt + dn
                        if dn == 0:
                            nc.gpsimd.dma_start(im[r:r + 1, :, 1:N],
                                                xd[:, ts, 0:N - 1])
                        elif dn == 1:
                            nc.gpsimd.dma_start(im[r:r + 1, :, :],
                                                xd[:, ts, :])
                        else:
                            nc.gpsimd.dma_start(im[r:r + 1, :, 0:N - 1],
                                                xd[:, ts, 1:N])
                s = tt % 3
                for p in range(PAIRS):
                    ps = pp.tile([128, 2, N], F32, tag="pair")
                    nc.tensor.matmul(ps[0:F1], w1[:], pr(im, p),
                                     start=True, stop=True)
                    nc.scalar.activation(hconv[:, s, 2 * p:2 * p + 2, 1:N + 1],
                                         ps[0:F1], AF.Identity,
                                         bias=c1b[:], scale=1.0)

            def conv2_l2_hn(t):
                """conv2 + bias, L2-normalize over (N,C) per (b,t), write hn."""
                g2v = io.tile([F1, BL, N], F32, tag="g2")
                for p in range(PAIRS):
                    ps = pp.tile([128, 2, N], F32, tag="pair")
                    for i9 in range(9):
                        dt, dn = divmod(i9, 3)
                        s = (t + dt - 1) % 3
                        nc.tensor.matmul(
                            ps[0:F1], w2[:, i9, :],
                            hconv[:, s, 2 * p:2 * p + 2, dn:dn + N],
                            start=(i9 == 0), stop=(i9 == 8))
                    nc.scalar.activation(pr(g2v, p), ps[0:F1], AF.Identity,
                                         bias=c2b[:], scale=1.0)
                sq = io.tile([F1, BL, N], F32, tag="sq")
                nc.vector.tensor_mul(sq[:], g2v[:], g2v[:])
                ns = scp.tile([F1, BL], F32, tag="ns")
                nc.vector.tensor_reduce(ns[:], sq[:], AX.X, Op.add)
                psS = pS.tile([128, BL], F32, tag="S")
                nc.tensor.matmul(psS[:], ones[0:F1, :], ns[:],
                                 start=True, stop=True)
                srt = scp.tile([128, BL], F32, tag="srt")
                nc.scalar.activation(srt[:], psS[:], AF.Sqrt,
                                     bias=eps[:], scale=1.0)
                scl = scp.tile([128, BL], F32, tag="scl")
                nc.vector.reciprocal(scl[:], srt[:])
                hn = hnt[t % 2]
                nc.vector.tensor_mul(hn[:, :, 1:N + 1], g2v[:],
                                     scl[0:F1, :].to_broadcast((F1, BL, N)))
                return hn

            def lstm_step(t, F, G, kx, kh, xin, hp, ct, gtiles, addc, gc_g):
                """One ConvLSTM1D step. gtiles: list of gate tiles, one per
                128-partition group. gc_g: group index holding tanh-gate.

                Gate nonlinearity: scalar engine does the hard-sigmoid affine
                (0.2x + 0.2b + 0.5) straight out of PSUM; vector engine then
                clips the FULL gate tile with one fused (max,min) tensor_scalar.
                Cell updates are full-tile ops (all 8 batches at once)."""
                ngrp = 4 * F // 128
                for g in range(ngrp):
                    gs = slice(128 * g, 128 * (g + 1))
                    for p in range(PAIRS):
                        ps = pp.tile([128, 2, N], F32, tag="pair")
                        for k in range(3):
                            nc.tensor.matmul(ps[:], kh[:, k, gs],
                                             hp[:, 2 * p:2 * p + 2, k:k + N],
                                             start=(k == 0), stop=False)
                        for k in range(3):
                            nc.tensor.matmul(ps[:], kx[:, k, gs],
                                             xin[:, 2 * p:2 * p + 2, k:k + N],
                                             start=False, stop=(k == 2))
                        gv = gtiles[g]
                        if F == 128:
                            if g == gc_g:
                                nc.scalar.activation(pr(gv, p), ps[:], AF.Tanh,
                                                     bias=addc[:, g:g + 1],
                                                     scale=1.0)
                            else:
                                nc.scalar.activation(pr(gv, p), ps[:],
                                                     AF.Identity,
                                                     bias=addc[:, g:g + 1],
                                                     scale=0.2)
                        else:
                            # F=64: g0 = i|f (both hard-sigmoid),
                            #       g1 = gc|o (tanh rows 0:64, hs rows 64:128)
                            if g == 0:
                                nc.scalar.activation(pr(gv, p), ps[:],
                                                     AF.Identity,
                                                     bias=addc[:, 0:1],
                                                     scale=0.2)
                            else:
                                nc.scalar.activation(
                                    pr(gv, p)[0:64], ps[0:64], AF.Tanh,
                                    bias=addc[0:64, 1:2], scale=1.0)
                                nc.scalar.activation(
                                    pr(gv, p)[64:128], ps[64:128],
                                    AF.Identity,
                                    bias=addc[64:128, 1:2], scale=0.2)
                # full-tile clips on the hard-sigmoid gates (vector TS max,min)
                if F == 128:
                    for g in range(ngrp):
                        if g != gc_g:
                            nc.vector.tensor_scalar(
                                out=gtiles[g][:], in0=gtiles[g][:],
                                scalar1=0.0, scalar2=1.0,
                                op0=Op.max, op1=Op.min)
                else:
                    nc.vector.tensor_scalar(
                        out=gtiles[0][:], in0=gtiles[0][:],
                        scalar1=0.0, scalar2=1.0, op0=Op.max, op1=Op.min)
                    nc.vector.tensor_scalar(
                        out=gtiles[1][64:128], in0=gtiles[1][64:128],
                        scalar1=0.0, scalar2=1.0, op0=Op.max, op1=Op.min)
                # full-tile cell update, in-place into the gate tiles:
                #   f-tile <- f*c ; i-tile <- i*gc ; c <- sum ; gc-tile <-
                #   tanh(c) ; h <- o * gc-tile
                if F == 128:
                    iv, fv = gtiles[0][:], gtiles[1][:]
                    gcv, ov = gtiles[2][:], gtiles[3][:]
                else:
                    iv = gtiles[0][0:64]
                    gcv = gtiles[1][0:64]
                    # f and o live on partitions 64:128; DMA-copy down
                    fvt = scp.tile([64, BL, N], F32, tag="fc")
                    nc.sync.dma_start(fvt[:], gtiles[0][64:128])
                    ovt = scp.tile([64, BL, N], F32, tag="oc")
                    nc.sync.dma_start(ovt[:], gtiles[1][64:128])
                    fv, ov = fvt[:], ovt[:]
                nc.vector.tensor_mul(fv, fv, ct[:])
                nc.vector.tensor_mul(iv, iv, gcv)
                nc.vector.tensor_add(ct[:], fv, iv)
                nc.scalar.activation(gcv, ct[:], AF.Tanh,
                                     bias=zero[0:F, :], scale=1.0)
                nc.vector.tensor_mul(hp[:, :, 1:N + 1], ov, gcv)

            # ---------------- main time loop ----------------
            conv1_time(0)
            for t in range(T):
                if t + 1 < T:
                    conv1_time(t + 1)
                else:
                    nc.vector.memset(hconv[:, (t + 1) % 3, :, :], 0.0)
                hn = conv2_l2_hn(t)
                lstm_step(t, F1, 2, kx1, kh1, hn, h1p, c1t, [G0, G1], addc1, 1)
                lstm_step(t, F2, 4, kx2, kh2, h1p, h2p, c2t, G2, addc2, 2)

            # ---------------- final L2 norm + FC ----------------
            sq2 = io.tile([F2, BL, N], F32, tag="sq2")
            nc.vector.tensor_mul(sq2[:], h2p[:, :, 1:N + 1], h2p[:, :, 1:N + 1])
            ns2 = scp.tile([F2, BL], F32, tag="ns")
            nc.vector.tensor_reduce(ns2[:], sq2[:], AX.X, Op.add)
            psS2 = pS.tile([128, BL], F32, tag="S")
            nc.tensor.matmul(psS2[:], ones[:], ns2[:], start=True, stop=True)
            srt2 = scp.tile([128, BL], F32, tag="srt")
            nc.scalar.activation(srt2[:], psS2[:], AF.Sqrt, bias=eps[:], scale=1.0)
            scl2 = scp.tile([128, BL], F32, tag="scl")
            nc.vector.reciprocal(scl2[:], srt2[:])
            ysb = io.tile([1, BL, N], F32, tag="y")
            for p in range(PAIRS):
                psY = pp.tile([128, 2, N], F32, tag="pair")
                nc.tensor.matmul(psY[0:1], fcw[:], h2p[:, 2 * p:2 * p + 2, 1:N + 1],
                                 start=True, stop=True)
                nc.vector.tensor_mul(pr(ysb, p), psY[0:1],
                                     scl2[0:1, 2 * p:2 * p + 2]
                                     .to_broadcast((1, 2, N)))
                nc.vector.tensor_scalar_add(pr(ysb, p), pr(ysb, p),
                                            fcb[0:1, 0:1])
            nc.sync.dma_start(y_d.rearrange("(u b) n -> u b n", u=1), ysb[:])

    from concourse import mybir as _mybir
    _legalize_waits(nc, _mybir)
    return nc


def kernel(**inputs):
    from concourse.bass_utils import run_bass_kernel_spmd

    if "nc" not in _CACHE:
        _CACHE["nc"] = _build_nc()
    nc = _CACHE["nc"]

    x = np.ascontiguousarray(np.asarray(inputs["x"], dtype=np.float32))
    shared = {k: np.ascontiguousarray(np.asarray(v, dtype=np.float32))
              for k, v in inputs.items() if k != "x"}
    shared["fc_w"] = shared["fc_w"].reshape(F2, 1)
    in_maps = []
    for c in range(NCORES):
        m = dict(shared)
        m["x"] = x[c * BL:(c + 1) * BL]
        in_maps.append(m)

    res = run_bass_kernel_spmd(nc, in_maps, core_ids=list(range(NCORES)))
    y = np.concatenate([res.results[c]["y"] for c in range(NCORES)], axis=0)
    return y.reshape(B_FULL, 1, N, 1).astype(np.float32)



# revision 9
# speedup vs baseline: 2.3340x; 1.2757x over previous
"""Trainium2 Bass kernel for nn_Conv2D_ConvLSTM1D (Conv2D stack -> 2x ConvLSTM1D -> FC).

Sharding: data-parallel over batch. 64 batches / 8 cores = 8 per core.
Each core runs the identical program on its batch shard; no collectives.

Per-core layout: channels on SBUF partitions, (batch, station) on the free
dim.  All convolutions become PE matmuls: contract dim = input channels
(or 3x3 taps for conv1), shifts over stations/time handled by sliding AP
windows into zero-padded SBUF tiles.  The whole network runs per-time-step
so every intermediate stays SBUF-resident (no DRAM spill):

  t-loop:  conv1(t+1) -> conv2(t) -> L2norm -> [LSTM1 gates/cell]
           -> [LSTM2 gates/cell]   (LSTM x-convs accumulate into the same
           PSUM tiles as the recurrent convs)

Matmul operands are bf16 (fp32 PSUM accumulation, fp32 cell state and gate
math) - measured end-to-end relative error vs the fp32 reference ~4e-3.
"""
import sys

if "/opt/trn_rl_repo" not in sys.path:
    sys.path.insert(0, "/opt/trn_rl_repo")

import numpy as np

B_FULL, T, N, CIN = 64, 24, 256, 1
F1, F2 = 64, 128
NCORES = 8
BL = B_FULL // NCORES          # batch per core
PAIRS = BL // 2                # matmuls process 2 batches (512 cols) at once

_CACHE = {}


def _legalize_waits(nc, mybir):
    """TRN2 engine instructions carry at most 1 sync wait (EventSemaphore: 2).
    Tile emits more; split extras onto injected NoOps placed just before."""
    n = 0
    for _, bbobj in list(nc.bb_map.items()):
        bb = bbobj.bb if hasattr(bbobj, "bb") else bbobj
        insts = bb.instructions
        newlist = []
        for inst in insts:
            si = getattr(inst, "sync_info", None)
            waits = list(si.on_wait) if (si is not None and si.on_wait) else []
            cap = 2 if isinstance(inst, mybir.InstEventSemaphore) else 1
            if len(waits) > cap:
                extra, keep = waits[:-cap], waits[-cap:]
                for w in extra:
                    n += 1
                    nop = mybir.InstNoOp(name=f"I-wf{n}", ins=[], outs=[])
                    nop.engine = inst.engine
                    nop.sync_info = mybir.SyncInfo(on_wait=[w], on_update=[])
                    nc.inst_map[nop.name] = nop
                    newlist.append(nop)
                inst.sync_info = mybir.SyncInfo(
                    on_wait=keep,
                    on_update=list(si.on_update) if si.on_update else [])
            newlist.append(inst)
        insts[:] = newlist
    return n


def _build_nc():
    import concourse.bass as bass
    import concourse.tile as tile
    from concourse import mybir
    from concourse.alu_op_type import AluOpType as Op

    F32 = mybir.dt.float32
    BF16 = mybir.dt.bfloat16
    AF = mybir.ActivationFunctionType
    AX = mybir.AxisListType

    nc = bass.Bass()

    x_d = nc.dram_tensor("x", [BL, T, N, CIN], F32, kind="ExternalInput")
    c1w_d = nc.dram_tensor("conv1_w", [3, 3, CIN, F1], F32, kind="ExternalInput")
    c1b_d = nc.dram_tensor("conv1_b", [F1], F32, kind="ExternalInput")
    c2w_d = nc.dram_tensor("conv2_w", [3, 3, F1, F1], F32, kind="ExternalInput")
    c2b_d = nc.dram_tensor("conv2_b", [F1], F32, kind="ExternalInput")
    kx1_d = nc.dram_tensor("kx1", [3, F1, 4 * F1], F32, kind="ExternalInput")
    kh1_d = nc.dram_tensor("kh1", [3, F1, 4 * F1], F32, kind="ExternalInput")
    b1_d = nc.dram_tensor("b1", [4 * F1], F32, kind="ExternalInput")
    kx2_d = nc.dram_tensor("kx2", [3, F1, 4 * F2], F32, kind="ExternalInput")
    kh2_d = nc.dram_tensor("kh2", [3, F2, 4 * F2], F32, kind="ExternalInput")
    b2_d = nc.dram_tensor("b2", [4 * F2], F32, kind="ExternalInput")
    fcw_d = nc.dram_tensor("fc_w", [F2, 1], F32, kind="ExternalInput")
    fcb_d = nc.dram_tensor("fc_b", [1], F32, kind="ExternalInput")
    y_d = nc.dram_tensor("y", [BL, N], F32, kind="ExternalOutput")

    xd = x_d.rearrange("b t n c -> b t (n c)")  # [BL, T, N]

    with tile.TileContext(nc) as tc:
        with (
            tc.tile_pool(name="wp", bufs=1) as wp,
            tc.tile_pool(name="st", bufs=1) as st,
            tc.tile_pool(name="io", bufs=2) as io,
            tc.tile_pool(name="gt", bufs=1) as gt,
            tc.tile_pool(name="sc", bufs=2) as scp,
            tc.tile_pool(name="pp", bufs=6, space="PSUM") as pp,
            tc.tile_pool(name="pS", bufs=2, space="PSUM") as pS,
        ):
            # ---------------- setup: weights (bf16 via casting gpsimd DMA) ----
            w1 = wp.tile([9, F1], BF16)
            nc.gpsimd.dma_start(w1[:], c1w_d.rearrange("a b c d -> (a b c) d"))
            w2 = wp.tile([F1, 9, F1], BF16)
            nc.gpsimd.dma_start(w2[:], c2w_d.rearrange("a b c d -> c (a b) d"))
            kx1 = wp.tile([F1, 3, 4 * F1], BF16)
            nc.gpsimd.dma_start(kx1[:], kx1_d.rearrange("k c f -> c k f"))
            kh1 = wp.tile([F1, 3, 4 * F1], BF16)
            nc.gpsimd.dma_start(kh1[:], kh1_d.rearrange("k c f -> c k f"))
            kx2 = wp.tile([F1, 3, 4 * F2], BF16)
            nc.gpsimd.dma_start(kx2[:], kx2_d.rearrange("k c f -> c k f"))
            kh2 = wp.tile([F2, 3, 4 * F2], BF16)
            nc.gpsimd.dma_start(kh2[:], kh2_d.rearrange("k c f -> c k f"))
            fcw = wp.tile([F2, 1], BF16)
            nc.gpsimd.dma_start(fcw[:], fcw_d[:])

            # biases / consts (fp32)
            c1b = wp.tile([F1, 1], F32)
            nc.sync.dma_start(c1b[:], c1b_d.rearrange("(f u) -> f u", u=1))
            c2b = wp.tile([F1, 1], F32)
            nc.sync.dma_start(c2b[:], c2b_d.rearrange("(f u) -> f u", u=1))
            b1s = wp.tile([128, 2], F32)
            nc.sync.dma_start(b1s[:], b1_d.rearrange("(g p) -> p g", g=2))
            b2s = wp.tile([128, 4], F32)
            nc.sync.dma_start(b2s[:], b2_d.rearrange("(g p) -> p g", g=4))
            fcb = wp.tile([1, 1], F32)
            nc.sync.dma_start(fcb[:], fcb_d.rearrange("(f u) -> f u", u=1))

            # hard-sigmoid-folded additive consts:
            #   i/f/o gates: gate = clip(0.2*psum + (0.2*b + 0.5), 0, 1)
            #   gc gate:     gate = tanh(psum + b)
            addc1 = wp.tile([128, 2], F32)
            nc.vector.tensor_scalar(out=addc1[:, 0:1], in0=b1s[:, 0:1],
                                    scalar1=0.2, scalar2=0.5,
                                    op0=Op.mult, op1=Op.add)
            nc.vector.tensor_copy(addc1[0:64, 1:2], b1s[0:64, 1:2])  # gc rows
            nc.vector.tensor_scalar(out=addc1[64:128, 1:2], in0=b1s[64:128, 1:2],
                                    scalar1=0.2, scalar2=0.5,
                                    op0=Op.mult, op1=Op.add)          # o rows
            addc2 = wp.tile([128, 4], F32)
            nc.vector.tensor_scalar(out=addc2[:], in0=b2s[:],
                                    scalar1=0.2, scalar2=0.5,
                                    op0=Op.mult, op1=Op.add)
            nc.vector.tensor_copy(addc2[:, 2:3], b2s[:, 2:3])         # gc group
            eps = wp.tile([128, 1], F32)
            nc.vector.memset(eps[:], 1e-12)
            zero = wp.tile([128, 1], F32)
            nc.vector.memset(zero[:], 0.0)
            ones = wp.tile([128, 128], F32)
            nc.vector.memset(ones[:], 1.0)

            # ---------------- persistent state ----------------
            hconv = st.tile([F1, 3, BL, N + 2], BF16)   # conv1 out, 3-slot t-window
            nc.vector.memset(hconv[:], 0.0)
            h1p = st.tile([F1, BL, N + 2], BF16)        # LSTM1 hidden (padded)
            nc.vector.memset(h1p[:], 0.0)
            c1t = st.tile([F1, BL, N], F32)             # LSTM1 cell
            nc.vector.memset(c1t[:], 0.0)
            h2p = st.tile([F2, BL, N + 2], BF16)        # LSTM2 hidden (padded)
            nc.vector.memset(h2p[:], 0.0)
            c2t = st.tile([F2, BL, N], F32)             # LSTM2 cell
            nc.vector.memset(c2t[:], 0.0)

            # double-buffered im2col / hn with persistent zero edges
            imt = [st.tile([9, BL, N], BF16, tag=f"im{i}", name=f"im{i}") for i in range(2)]
            for tl in imt:
                nc.vector.memset(tl[:], 0.0)
            hnt = [st.tile([F1, BL, N + 2], BF16, tag=f"hn{i}", name=f"hn{i}") for i in range(2)]
            for tl in hnt:
                nc.vector.memset(tl[:], 0.0)

            # gate tiles (full batch width)
            G0 = gt.tile([128, BL, N], F32, tag="G0")   # L1: i|f
            G1 = gt.tile([128, BL, N], F32, tag="G1")   # L1: gc|o
            G2 = [gt.tile([128, BL, N], F32, tag=f"L2g{g}", name=f"L2g{g}") for g in range(4)]

            def pr(a, p):  # batch-pair slice helper on dim after partitions
                return a[:, 2 * p:2 * p + 2, :]

            # ------------- per-time-step pieces -------------
            def conv1_time(tt):
                """im2col DMA + matmul; writes hconv slot tt%3 (bias folded)."""
                im = imt[tt % 2]
                if tt == 0 or tt == T - 1:
                    # t-edge: some taps fall outside [0,T); clear whole tile
                    # (partition-base of row-range memsets must be 32-aligned)
                    nc.vector.memset(im[:], 0.0)
                for dt in range(3):
                    ts = tt + dt - 1
                    if not (0 <= ts < T):
                        continue
                    for dn in range(3):
                        r = 3 * dt + dn
                        if dn == 0:
                            nc.gpsimd.dma_start(im[r:r + 1, :, 1:N],
                                                xd[:, ts, 0:N - 1])
                        elif dn == 1:
                            nc.gpsimd.dma_start(im[r:r + 1, :, :],
                                                xd[:, ts, :])
                        else:
                            nc.gpsimd.dma_start(im[r:r + 1, :, 0:N - 1],
                                                xd[:, ts, 1:N])
                s = tt % 3
                for p in range(PAIRS):
                    ps = pp.tile([128, 2, N], F32, tag="pair")
                    nc.tensor.matmul(ps[0:F1], w1[:], pr(im, p),
                                     start=True, stop=True)
                    nc.scalar.activation(hconv[:, s, 2 * p:2 * p + 2, 1:N + 1],
                                         ps[0:F1], AF.Identity,
                                         bias=c1b[:], scale=1.0)

            def conv2_l2_hn(t):
                """conv2 + bias, L2-normalize over (N,C) per (b,t), write hn.
                Tap-outer / pair-inner so 4 consecutive matmuls share the
                stationary weight (amortizes LDWEIGHTS)."""
                g2v = io.tile([F1, BL, N], F32, tag="g2")
                pst = [pp.tile([128, 2, N], F32, tag="pair", name=f"c2ps{_p}") for _p in range(PAIRS)]
                for i9 in range(9):
                    dt, dn = divmod(i9, 3)
                    s = (t + dt - 1) % 3
                    for p in range(PAIRS):
                        nc.tensor.matmul(
                            pst[p][0:F1], w2[:, i9, :],
                            hconv[:, s, 2 * p:2 * p + 2, dn:dn + N],
                            start=(i9 == 0), stop=(i9 == 8))
                for p in range(PAIRS):
                    nc.scalar.activation(pr(g2v, p), pst[p][0:F1], AF.Identity,
                                         bias=c2b[:], scale=1.0)
                return g2v

            def l2n_hn(t, g2v):
                """L2-normalize conv2 output over (N,C) per (b,t), write hn."""
                sq = io.tile([F1, BL, N], F32, tag="sq")
                nc.scalar.activation(sq[:], g2v[:], AF.Square,
                                     bias=zero[0:F1, :], scale=1.0)
                ns = scp.tile([F1, BL], F32, tag="ns")
                nc.vector.tensor_reduce(ns[:], sq[:], AX.X, Op.add)
                psS = pS.tile([128, BL], F32, tag="S")
                nc.tensor.matmul(psS[:], ones[0:F1, :], ns[:],
                                 start=True, stop=True)
                srt = scp.tile([128, BL], F32, tag="srt")
                nc.scalar.activation(srt[:], psS[:], AF.Sqrt,
                                     bias=eps[:], scale=1.0)
                scl = scp.tile([128, BL], F32, tag="scl")
                nc.vector.reciprocal(scl[:], srt[:])
                hn = hnt[t % 2]
                nc.vector.tensor_mul(hn[:, :, 1:N + 1], g2v[:],
                                     scl[0:F1, :].to_broadcast((F1, BL, N)))
                return hn

            def lstm_step(t, F, G, kx, kh, xin, hp, ct, gtiles, addc, gc_g):
                """One ConvLSTM1D step. gtiles: list of gate tiles, one per
                128-partition group. gc_g: group index holding tanh-gate.

                Gate nonlinearity: scalar engine does the hard-sigmoid affine
                (0.2x + 0.2b + 0.5) straight out of PSUM; vector engine then
                clips the FULL gate tile with one fused (max,min) tensor_scalar.
                Cell updates are full-tile ops (all 8 batches at once)."""
                ngrp = 4 * F // 128
                for g in range(ngrp):
                    gs = slice(128 * g, 128 * (g + 1))
                    pst = [pp.tile([128, 2, N], F32, tag="pair",
                                    name=f"g{g}ps{_p}") for _p in range(PAIRS)]
                    for k in range(3):
                        for p in range(PAIRS):
                            nc.tensor.matmul(pst[p][:], kh[:, k, gs],
                                             hp[:, 2 * p:2 * p + 2, k:k + N],
                                             start=(k == 0), stop=False)
                    for k in range(3):
                        for p in range(PAIRS):
                            nc.tensor.matmul(pst[p][:], kx[:, k, gs],
                                             xin[:, 2 * p:2 * p + 2, k:k + N],
                                             start=False, stop=(k == 2))
                    for p in range(PAIRS):
                        ps = pst[p]
                        gv = gtiles[g]
                        if F == 128:
                            if g == gc_g:
                                nc.scalar.activation(pr(gv, p), ps[:], AF.Tanh,
                                                     bias=addc[:, g:g + 1],
                                                     scale=1.0)
                            else:
                                nc.scalar.activation(pr(gv, p), ps[:],
                                                     AF.Identity,
                                                     bias=addc[:, g:g + 1],
                                                     scale=0.2)
                        else:
                            # F=64: g0 = i|f (both hard-sigmoid),
                            #       g1 = gc|o (tanh rows 0:64, hs rows 64:128)
                            if g == 0:
                                nc.scalar.activation(pr(gv, p), ps[:],
                                                     AF.Identity,
                                                     bias=addc[:, 0:1],
                                                     scale=0.2)
                            else:
                                nc.scalar.activation(
                                    pr(gv, p)[0:64], ps[0:64], AF.Tanh,
                                    bias=addc[0:64, 1:2], scale=1.0)
                                nc.scalar.activation(
                                    pr(gv, p)[64:128], ps[64:128],
                                    AF.Identity,
                                    bias=addc[64:128, 1:2], scale=0.2)
                # full-tile clips on the hard-sigmoid gates (vector TS max,min)
                if F == 128:
                    for g in range(ngrp):
                        if g != gc_g:
                            nc.vector.tensor_scalar(
                                out=gtiles[g][:], in0=gtiles[g][:],
                                scalar1=0.0, scalar2=1.0,
                                op0=Op.max, op1=Op.min)
                else:
                    nc.vector.tensor_scalar(
                        out=gtiles[0][:], in0=gtiles[0][:],
                        scalar1=0.0, scalar2=1.0, op0=Op.max, op1=Op.min)
                    nc.vector.tensor_scalar(
                        out=gtiles[1][64:128], in0=gtiles[1][64:128],
                        scalar1=0.0, scalar2=1.0, op0=Op.max, op1=Op.min)
                # full-tile cell update, in-place into the gate tiles:
                #   f-tile <- f*c ; i-tile <- i*gc ; c <- sum ; gc-tile <-
                #   tanh(c) ; h <- o * gc-tile
                if F == 128:
                    iv, fv = gtiles[0][:], gtiles[1][:]
                    gcv, ov = gtiles[2][:], gtiles[3][:]
                else:
                    iv = gtiles[0][0:64]
                    gcv = gtiles[1][0:64]
                    # f and o live on partitions 64:128; DMA-copy down
                    fvt = scp.tile([64, BL, N], F32, tag="fc")
                    nc.sync.dma_start(fvt[:], gtiles[0][64:128])
                    ovt = scp.tile([64, BL, N], F32, tag="oc")
                    nc.sync.dma_start(ovt[:], gtiles[1][64:128])
                    fv, ov = fvt[:], ovt[:]
                nc.vector.tensor_mul(fv, fv, ct[:])
                nc.vector.tensor_mul(iv, iv, gcv)
                nc.vector.tensor_add(ct[:], fv, iv)
                nc.scalar.activation(gcv, ct[:], AF.Tanh,
                                     bias=zero[0:F, :], scale=1.0)
                nc.vector.tensor_mul(hp[:, :, 1:N + 1], ov, gcv)

            # ---------------- main time loop ----------------
            # Software-pipelined: iteration t emits conv1(t+2) and the
            # conv2(t+1) matmuls up front (PE work to fill the serial
            # vector window of step t), then the t-step LSTM work, then the
            # L2-norm tail of t+1 (whose S-matmul must not block the LSTM
            # matmuls in the PE FIFO).
            conv1_time(0)
            conv1_time(1)
            hn_cur = l2n_hn(0, conv2_l2_hn(0))
            for t in range(T):
                if t + 2 < T:
                    conv1_time(t + 2)
                elif t + 2 == T:
                    nc.vector.memset(hconv[:, T % 3, :, :], 0.0)
                g2v_next = conv2_l2_hn(t + 1) if t + 1 < T else None
                lstm_step(t, F1, 2, kx1, kh1, hn_cur, h1p, c1t, [G0, G1],
                          addc1, 1)
                lstm_step(t, F2, 4, kx2, kh2, h1p, h2p, c2t, G2, addc2, 2)
                if g2v_next is not None:
                    hn_cur = l2n_hn(t + 1, g2v_next)

            # ---------------- final L2 norm + FC ----------------
            sq2 = io.tile([F2, BL, N], F32, tag="sq2")
            nc.vector.tensor_mul(sq2[:], h2p[:, :, 1:N + 1], h2p[:, :, 1:N + 1])
            ns2 = scp.tile([F2, BL], F32, tag="ns")
            nc.vector.tensor_reduce(ns2[:], sq2[:], AX.X, Op.add)
            psS2 = pS.tile([128, BL], F32, tag="S")
            nc.tensor.matmul(psS2[:], ones[:], ns2[:], start=True, stop=True)
            srt2 = scp.tile([128, BL], F32, tag="srt")
            nc.scalar.activation(srt2[:], psS2[:], AF.Sqrt, bias=eps[:], scale=1.0)
            scl2 = scp.tile([128, BL], F32, tag="scl")
            nc.vector.reciprocal(scl2[:], srt2[:])
            ysb = io.tile([1, BL, N], F32, tag="y")
            for p in range(PAIRS):
                psY = pp.tile([128, 2, N], F32, tag="pair")
                nc.tensor.matmul(psY[0:1], fcw[:], h2p[:, 2 * p:2 * p + 2, 1:N + 1],
                                 start=True, stop=True)
                nc.vector.tensor_mul(pr(ysb, p), psY[0:1],
                                     scl2[0:1, 2 * p:2 * p + 2]
                                     .to_broadcast((1, 2, N)))
                nc.vector.tensor_scalar_add(pr(ysb, p), pr(ysb, p),
                                            fcb[0:1, 0:1])
            nc.sync.dma_start(y_d.rearrange("(u b) n -> u b n", u=1), ysb[:])

    from concourse import mybir as _mybir
    _legalize_waits(nc, _mybir)
    return nc


def kernel(**inputs):
    from concourse.bass_utils import run_bass_kernel_spmd

    if "nc" not in _CACHE:
        _CACHE["nc"] = _build_nc()
    nc = _CACHE["nc"]

    x = np.ascontiguousarray(np.asarray(inputs["x"], dtype=np.float32))
    shared = {k: np.ascontiguousarray(np.asarray(v, dtype=np.float32))
              for k, v in inputs.items() if k != "x"}
    shared["fc_w"] = shared["fc_w"].reshape(F2, 1)
    in_maps = []
    for c in range(NCORES):
        m = dict(shared)
        m["x"] = x[c * BL:(c + 1) * BL]
        in_maps.append(m)

    res = run_bass_kernel_spmd(nc, in_maps, core_ids=list(range(NCORES)))
    y = np.concatenate([res.results[c]["y"] for c in range(NCORES)], axis=0)
    return y.reshape(B_FULL, 1, N, 1).astype(np.float32)



# revision 18
# speedup vs baseline: 2.6774x; 1.1471x over previous
"""Trainium2 Bass kernel for nn_Conv2D_ConvLSTM1D (Conv2D stack -> 2x ConvLSTM1D -> FC).

Sharding: data-parallel over batch. 64 batches / 8 cores = 8 per core.
Each core runs the identical program on its batch shard; no collectives.

Per-core layout: channels on SBUF partitions, (batch, station) on the free
dim.  All convolutions become PE matmuls: contract dim = input channels
(or 3x3 taps for conv1), shifts over stations/time handled by sliding AP
windows into zero-padded SBUF tiles.  The whole network runs per-time-step
so every intermediate stays SBUF-resident (no DRAM spill):

  t-loop:  conv1(t+1) -> conv2(t) -> L2norm -> [LSTM1 gates/cell]
           -> [LSTM2 gates/cell]   (LSTM x-convs accumulate into the same
           PSUM tiles as the recurrent convs)

Matmul operands are bf16 (fp32 PSUM accumulation, fp32 cell state and gate
math) - measured end-to-end relative error vs the fp32 reference ~4e-3.
"""
import sys

if "/opt/trn_rl_repo" not in sys.path:
    sys.path.insert(0, "/opt/trn_rl_repo")

import numpy as np

B_FULL, T, N, CIN = 64, 24, 256, 1
F1, F2 = 64, 128
NCORES = 8
BL = B_FULL // NCORES          # batch per core
PAIRS = BL // 2                # matmuls process 2 batches (512 cols) at once

_CACHE = {}


def _legalize_waits(nc, mybir):
    """TRN2 engine instructions carry at most 1 sync wait (EventSemaphore: 2).
    Tile emits more; split extras onto injected NoOps placed just before."""
    n = 0
    for _, bbobj in list(nc.bb_map.items()):
        bb = bbobj.bb if hasattr(bbobj, "bb") else bbobj
        insts = bb.instructions
        newlist = []
        for inst in insts:
            si = getattr(inst, "sync_info", None)
            waits = list(si.on_wait) if (si is not None and si.on_wait) else []
            cap = 2 if isinstance(inst, mybir.InstEventSemaphore) else 1
            if len(waits) > cap:
                extra, keep = waits[:-cap], waits[-cap:]
                for w in extra:
                    n += 1
                    nop = mybir.InstNoOp(name=f"I-wf{n}", ins=[], outs=[])
                    nop.engine = inst.engine
                    nop.sync_info = mybir.SyncInfo(on_wait=[w], on_update=[])
                    nc.inst_map[nop.name] = nop
                    newlist.append(nop)
                inst.sync_info = mybir.SyncInfo(
                    on_wait=keep,
                    on_update=list(si.on_update) if si.on_update else [])
            newlist.append(inst)
        insts[:] = newlist
    return n


def _build_nc():
    import concourse.bass as bass
    import concourse.tile as tile
    from concourse import mybir
    from concourse.alu_op_type import AluOpType as Op

    F32 = mybir.dt.float32
    BF16 = mybir.dt.bfloat16
    AF = mybir.ActivationFunctionType
    AX = mybir.AxisListType

    nc = bass.Bass()

    x_d = nc.dram_tensor("x", [BL, T, N, CIN], F32, kind="ExternalInput")
    c1w_d = nc.dram_tensor("conv1_w", [3, 3, CIN, F1], F32, kind="ExternalInput")
    c1b_d = nc.dram_tensor("conv1_b", [F1], F32, kind="ExternalInput")
    c2w_d = nc.dram_tensor("conv2_w", [3, 3, F1, F1], F32, kind="ExternalInput")
    c2b_d = nc.dram_tensor("conv2_b", [F1], F32, kind="ExternalInput")
    kx1_d = nc.dram_tensor("kx1", [3, F1, 4 * F1], F32, kind="ExternalInput")
    kh1_d = nc.dram_tensor("kh1", [3, F1, 4 * F1], F32, kind="ExternalInput")
    b1_d = nc.dram_tensor("b1", [4 * F1], F32, kind="ExternalInput")
    kx2_d = nc.dram_tensor("kx2", [3, F1, 4 * F2], F32, kind="ExternalInput")
    kh2_d = nc.dram_tensor("kh2", [3, F2, 4 * F2], F32, kind="ExternalInput")
    b2_d = nc.dram_tensor("b2", [4 * F2], F32, kind="ExternalInput")
    fcw_d = nc.dram_tensor("fc_w", [F2, 1], F32, kind="ExternalInput")
    fcb_d = nc.dram_tensor("fc_b", [1], F32, kind="ExternalInput")
    y_d = nc.dram_tensor("y", [BL, N], F32, kind="ExternalOutput")

    xd = x_d.rearrange("b t n c -> b t (n c)")  # [BL, T, N]

    with tile.TileContext(nc) as tc:
        with (
            tc.tile_pool(name="wp", bufs=1) as wp,
            tc.tile_pool(name="st", bufs=1) as st,
            tc.tile_pool(name="io", bufs=2) as io,
            tc.tile_pool(name="gt", bufs=1) as gt,
            tc.tile_pool(name="sc", bufs=2) as scp,
            tc.tile_pool(name="pp", bufs=8, space="PSUM") as pp,
        ):
            # ---------------- setup: weights (bf16 via casting gpsimd DMA) ----
            w1 = wp.tile([9, F1], BF16)
            nc.gpsimd.dma_start(w1[:], c1w_d.rearrange("a b c d -> (a b c) d"))
            w2 = wp.tile([F1, 9, F1], BF16)
            nc.gpsimd.dma_start(w2[:], c2w_d.rearrange("a b c d -> c (a b) d"))
            # L1 x|h packed weights: rows 0:64 = kx1, rows 64:128 = kh1 so
            # the gate conv is a single 128-contract matmul against the
            # combined [hn | h1] tile.
            kxh1 = wp.tile([128, 3, 4 * F1], BF16)
            nc.gpsimd.dma_start(kxh1[0:F1], kx1_d.rearrange("k c f -> c k f"))
            nc.gpsimd.dma_start(kxh1[F1:128], kh1_d.rearrange("k c f -> c k f"))
            # kx2 lives on rows 64:128 to match h1's partition rows in xh1
            kx2s = wp.tile([128, 3, 4 * F2], BF16)
            nc.gpsimd.dma_start(kx2s[64:128], kx2_d.rearrange("k c f -> c k f"))
            kh2 = wp.tile([F2, 3, 4 * F2], BF16)
            nc.gpsimd.dma_start(kh2[:], kh2_d.rearrange("k c f -> c k f"))
            fcw = wp.tile([F2, 1], BF16)
            nc.gpsimd.dma_start(fcw[:], fcw_d[:])

            # biases / consts (fp32)
            c1b = wp.tile([F1, 1], F32)
            nc.sync.dma_start(c1b[:], c1b_d.rearrange("(f u) -> f u", u=1))
            c2b = wp.tile([F1, 1], F32)
            nc.sync.dma_start(c2b[:], c2b_d.rearrange("(f u) -> f u", u=1))
            b1s = wp.tile([128, 2], F32)
            nc.sync.dma_start(b1s[:], b1_d.rearrange("(g p) -> p g", g=2))
            b2s = wp.tile([128, 4], F32)
            nc.sync.dma_start(b2s[:], b2_d.rearrange("(g p) -> p g", g=4))
            fcb = wp.tile([1, 1], F32)
            nc.sync.dma_start(fcb[:], fcb_d.rearrange("(f u) -> f u", u=1))

            # hard-sigmoid-folded additive consts:
            #   i/f/o gates: gate = clip(0.2*psum + (0.2*b + 0.5), 0, 1)
            #   gc gate:     gate = tanh(psum + b)
            addc1 = wp.tile([128, 2], F32)
            nc.vector.tensor_scalar(out=addc1[:, 0:1], in0=b1s[:, 0:1],
                                    scalar1=0.2, scalar2=0.5,
                                    op0=Op.mult, op1=Op.add)
            nc.vector.tensor_copy(addc1[0:64, 1:2], b1s[0:64, 1:2])  # gc rows
            nc.vector.tensor_scalar(out=addc1[64:128, 1:2], in0=b1s[64:128, 1:2],
                                    scalar1=0.2, scalar2=0.5,
                                    op0=Op.mult, op1=Op.add)          # o rows
            addc2 = wp.tile([128, 4], F32)
            nc.vector.tensor_scalar(out=addc2[:], in0=b2s[:],
                                    scalar1=0.2, scalar2=0.5,
                                    op0=Op.mult, op1=Op.add)
            nc.vector.tensor_copy(addc2[:, 2:3], b2s[:, 2:3])         # gc group
            eps = wp.tile([128, 1], F32)
            nc.vector.memset(eps[:], 1e-12)
            zero = wp.tile([128, 1], F32)
            nc.vector.memset(zero[:], 0.0)
            ones = wp.tile([128, 128], F32)
            nc.vector.memset(ones[:], 1.0)

            # ---------------- persistent state ----------------
            hconv = st.tile([F1, 3, BL, N + 2], BF16)   # conv1 out, 3-slot t-window
            nc.vector.memset(hconv[:], 0.0)
            # combined LSTM1 input: rows 0:64 = hn(t) (conv2 normalized),
            # rows 64:128 = h1(t-1); double-buffered over t parity
            xh1 = [st.tile([128, BL, N + 2], BF16, tag=f"xh{i}", name=f"xh{i}")
                   for i in range(2)]
            for tl in xh1:
                nc.vector.memset(tl[:], 0.0)
            c1t = st.tile([128, BL, N], F32)            # LSTM1 cell (rows 64:128)
            nc.vector.memset(c1t[:], 0.0)
            m2u = st.tile([128, BL, N], F32)            # i*gc moved to rows 64:128
            h2p = st.tile([F2, BL, N + 2], BF16)        # LSTM2 hidden (padded)
            nc.vector.memset(h2p[:], 0.0)
            c2t = st.tile([F2, BL, N], F32)             # LSTM2 cell
            nc.vector.memset(c2t[:], 0.0)

            # double-buffered im2col with persistent zero edges
            imt = [st.tile([9, BL, N], BF16, tag=f"im{i}", name=f"im{i}") for i in range(2)]
            for tl in imt:
                nc.vector.memset(tl[:], 0.0)

            # gate tiles (full batch width)
            G0 = gt.tile([128, BL, N], F32, tag="G0")   # L1: i|f
            G1 = gt.tile([128, BL, N], F32, tag="G1")   # L1: gc|o
            G2 = [gt.tile([128, BL, N], F32, tag=f"L2g{g}", name=f"L2g{g}") for g in range(4)]

            def pr(a, p):  # batch-pair slice helper on dim after partitions
                return a[:, 2 * p:2 * p + 2, :]

            # ------------- per-time-step pieces -------------
            def conv1_time(tt):
                """im2col DMA + matmul; writes hconv slot tt%3 (bias folded)."""
                im = imt[tt % 2]
                if tt == 0 or tt == T - 1:
                    # t-edge: some taps fall outside [0,T); clear whole tile
                    # (partition-base of row-range memsets must be 32-aligned)
                    nc.vector.memset(im[:], 0.0)
                for dt in range(3):
                    ts = tt + dt - 1
                    if not (0 <= ts < T):
                        continue
                    for dn in range(3):
                        r = 3 * dt + dn
                        if dn == 0:
                            nc.gpsimd.dma_start(im[r:r + 1, :, 1:N],
                                                xd[:, ts, 0:N - 1])
                        elif dn == 1:
                            nc.gpsimd.dma_start(im[r:r + 1, :, :],
                                                xd[:, ts, :])
                        else:
                            nc.gpsimd.dma_start(im[r:r + 1, :, 0:N - 1],
                                                xd[:, ts, 1:N])
                s = tt % 3
                for p in range(PAIRS):
                    ps = pp.tile([128, 2, N], F32, tag="pair")
                    nc.tensor.matmul(ps[0:F1], w1[:], pr(im, p),
                                     start=True, stop=True)
                    nc.scalar.activation(hconv[:, s, 2 * p:2 * p + 2, 1:N + 1],
                                         ps[0:F1], AF.Identity,
                                         bias=c1b[:], scale=1.0)

            def conv2_l2_hn(t):
                """conv2 + bias, L2-normalize over (N,C) per (b,t), write hn.
                Tap-outer / pair-inner so 4 consecutive matmuls share the
                stationary weight (amortizes LDWEIGHTS)."""
                g2v = io.tile([F1, BL, N], F32, tag="g2")
                pst = [pp.tile([128, 2, N], F32, tag="pair", name=f"c2ps{_p}") for _p in range(PAIRS)]
                for i9 in range(9):
                    dt, dn = divmod(i9, 3)
                    s = (t + dt - 1) % 3
                    for p in range(PAIRS):
                        nc.tensor.matmul(
                            pst[p][0:F1], w2[:, i9, :],
                            hconv[:, s, 2 * p:2 * p + 2, dn:dn + N],
                            start=(i9 == 0), stop=(i9 == 8))
                for p in range(PAIRS):
                    nc.scalar.activation(pr(g2v, p), pst[p][0:F1], AF.Identity,
                                         bias=c2b[:], scale=1.0)
                return g2v

            def l2n_hn(t, g2v):
                """L2-normalize conv2 output over (N,C) per (b,t), write hn."""
                sq = io.tile([F1, BL, N], F32, tag="sq")
                nc.scalar.activation(sq[:], g2v[:], AF.Square,
                                     bias=zero[0:F1, :], scale=1.0)
                ns = scp.tile([F1, BL], F32, tag="ns")
                nc.vector.tensor_reduce(ns[:], sq[:], AX.X, Op.add)
                psT = pp.tile([128, 2, N], F32, tag="pair", name="psS")
                psS = psT[:, 0, 0:BL]
                nc.tensor.matmul(psS, ones[0:F1, :], ns[:],
                                 start=True, stop=True)
                srt = scp.tile([128, BL], F32, tag="srt")
                nc.scalar.activation(srt[:], psS, AF.Sqrt,
                                     bias=eps[:], scale=1.0)
                scl = scp.tile([128, BL], F32, tag="scl")
                nc.vector.reciprocal(scl[:], srt[:])
                nc.vector.tensor_mul(xh1[t % 2][0:F1, :, 1:N + 1], g2v[:],
                                     scl[0:F1, :].to_broadcast((F1, BL, N)))

            def lstm1_step(t):
                """LSTM1 (F=64): x|h packed 128-contract gate matmuls from
                xh1[t%2]; writes h1(t) into xh1[(t+1)%2] rows 64:128.
                Gate layout: G0 = i|f, G1 = gc|o.  Cell state c1t lives on
                partitions 64:128 so m1=f*c, c, tanh(c), h all stay on the
                f/o rows; only m2=i*gc needs one cross-partition DMA."""
                xh = xh1[t % 2]
                xhn = xh1[(t + 1) % 2]
                for g in range(2):
                    gs = slice(128 * g, 128 * (g + 1))
                    pst = [pp.tile([128, 2, N], F32, tag="pair",
                                   name=f"l1g{g}ps{_p}") for _p in range(PAIRS)]
                    for k in range(3):
                        for p in range(PAIRS):
                            nc.tensor.matmul(pst[p][:], kxh1[:, k, gs],
                                             xh[:, 2 * p:2 * p + 2, k:k + N],
                                             start=(k == 0), stop=(k == 2))
                    for p in range(PAIRS):
                        ps = pst[p]
                        if g == 0:
                            nc.scalar.activation(pr(G0, p), ps[:],
                                                 AF.Identity,
                                                 bias=addc1[:, 0:1],
                                                 scale=0.2)
                        else:
                            nc.scalar.activation(
                                pr(G1, p)[0:64], ps[0:64], AF.Tanh,
                                bias=addc1[0:64, 1:2], scale=1.0)
                            nc.scalar.activation(
                                pr(G1, p)[64:128], ps[64:128],
                                AF.Identity,
                                bias=addc1[64:128, 1:2], scale=0.2)
                nc.vector.tensor_scalar(
                    out=G0[:], in0=G0[:],
                    scalar1=0.0, scalar2=1.0, op0=Op.max, op1=Op.min)
                nc.vector.tensor_scalar(
                    out=G1[64:128], in0=G1[64:128],
                    scalar1=0.0, scalar2=1.0, op0=Op.max, op1=Op.min)
                # cell update
                nc.vector.tensor_mul(G0[0:64], G0[0:64], G1[0:64])  # m2=i*gc
                nc.sync.dma_start(m2u[64:128], G0[0:64])
                nc.vector.tensor_mul(G0[64:128], G0[64:128],
                                     c1t[64:128])                   # m1=f*c
                nc.vector.tensor_add(c1t[64:128], G0[64:128], m2u[64:128])
                nc.scalar.activation(G0[64:128], c1t[64:128], AF.Tanh,
                                     bias=zero[64:128, :], scale=1.0)
                nc.vector.tensor_mul(xhn[64:128, :, 1:N + 1],
                                     G1[64:128], G0[64:128])

            def lstm2_step(t):
                """LSTM2 (F=128): gate groups g = i,f,gc,o.  h-conv matmuls
                (reading h2p, ready since t-1) are emitted ahead of the
                x-conv matmuls (which wait on h1(t)) so the PE has work
                while the LSTM1 cell chain completes.  x-conv reads h1 on
                rows 64:128 of xh1[(t+1)%2] with kx2 staged on rows 64:128."""
                xh = xh1[(t + 1) % 2]
                pst = {}

                def kh_g(g):
                    gs = slice(128 * g, 128 * (g + 1))
                    pst[g] = [pp.tile([128, 2, N], F32, tag="pair",
                                      name=f"l2g{g}ps{_p}")
                              for _p in range(PAIRS)]
                    for k in range(3):
                        for p in range(PAIRS):
                            nc.tensor.matmul(pst[g][p][:], kh2[:, k, gs],
                                             h2p[:, 2 * p:2 * p + 2, k:k + N],
                                             start=(k == 0), stop=False)

                def kx_g(g):
                    gs = slice(128 * g, 128 * (g + 1))
                    for k in range(3):
                        for p in range(PAIRS):
                            nc.tensor.matmul(
                                pst[g][p][:], kx2s[64:128, k, gs],
                                xh[64:128, 2 * p:2 * p + 2, k:k + N],
                                start=False, stop=(k == 2))

                def aff_g(g):
                    for p in range(PAIRS):
                        if g == 2:
                            nc.scalar.activation(pr(G2[g], p), pst[g][p][:],
                                                 AF.Tanh,
                                                 bias=addc2[:, g:g + 1],
                                                 scale=1.0)
                        else:
                            nc.scalar.activation(pr(G2[g], p), pst[g][p][:],
                                                 AF.Identity,
                                                 bias=addc2[:, g:g + 1],
                                                 scale=0.2)

                kh_g(0); kh_g(1); kx_g(0); aff_g(0)
                kh_g(2); kx_g(1); aff_g(1)
                kh_g(3); kx_g(2); aff_g(2)
                kx_g(3); aff_g(3)
                for g in (0, 1, 3):
                    nc.vector.tensor_scalar(
                        out=G2[g][:], in0=G2[g][:],
                        scalar1=0.0, scalar2=1.0, op0=Op.max, op1=Op.min)
                # cell update in-place: i<-i*gc, f<-f*c, c<-sum, gc<-tanh(c)
                nc.vector.tensor_mul(G2[1][:], G2[1][:], c2t[:])
                nc.vector.tensor_mul(G2[0][:], G2[0][:], G2[2][:])
                nc.vector.tensor_add(c2t[:], G2[1][:], G2[0][:])
                nc.scalar.activation(G2[2][:], c2t[:], AF.Tanh,
                                     bias=zero[:], scale=1.0)
                nc.vector.tensor_mul(h2p[:, :, 1:N + 1], G2[3][:], G2[2][:])

            # ---------------- main time loop ----------------
            # Software-pipelined: iteration t emits conv1(t+2) and the
            # conv2(t+1) matmuls up front (PE work to fill the serial
            # vector window of step t), then the t-step LSTM work, then the
            # L2-norm tail of t+1 (whose S-matmul must not block the LSTM
            # matmuls in the PE FIFO).
            conv1_time(0)
            conv1_time(1)
            l2n_hn(0, conv2_l2_hn(0))
            for t in range(T):
                if t + 2 < T:
                    conv1_time(t + 2)
                elif t + 2 == T:
                    nc.vector.memset(hconv[:, T % 3, :, :], 0.0)
                g2v_next = conv2_l2_hn(t + 1) if t + 1 < T else None
                lstm1_step(t)
                lstm2_step(t)
                if g2v_next is not None:
                    l2n_hn(t + 1, g2v_next)

            # ---------------- final L2 norm + FC ----------------
            sq2 = io.tile([F2, BL, N], F32, tag="sq2")
            nc.vector.tensor_mul(sq2[:], h2p[:, :, 1:N + 1], h2p[:, :, 1:N + 1])
            ns2 = scp.tile([F2, BL], F32, tag="ns")
            nc.vector.tensor_reduce(ns2[:], sq2[:], AX.X, Op.add)
            psT2 = pp.tile([128, 2, N], F32, tag="pair", name="psS2")
            psS2 = psT2[:, 0, 0:BL]
            nc.tensor.matmul(psS2, ones[:], ns2[:], start=True, stop=True)
            srt2 = scp.tile([128, BL], F32, tag="srt")
            nc.scalar.activation(srt2[:], psS2, AF.Sqrt, bias=eps[:], scale=1.0)
            scl2 = scp.tile([128, BL], F32, tag="scl")
            nc.vector.reciprocal(scl2[:], srt2[:])
            ysb = io.tile([1, BL, N], F32, tag="y")
            for p in range(PAIRS):
                psY = pp.tile([128, 2, N], F32, tag="pair")
                nc.tensor.matmul(psY[0:1], fcw[:], h2p[:, 2 * p:2 * p + 2, 1:N + 1],
                                 start=True, stop=True)
                nc.vector.tensor_mul(pr(ysb, p), psY[0:1],
                                     scl2[0:1, 2 * p:2 * p + 2]
                                     .to_broadcast((1, 2, N)))
                nc.vector.tensor_scalar_add(pr(ysb, p), pr(ysb, p),
                                            fcb[0:1, 0:1])
            nc.sync.dma_start(y_d.rearrange("(u b) n -> u b n", u=1), ysb[:])

    from concourse import mybir as _mybir
    _legalize_waits(nc, _mybir)
    return nc


def kernel(**inputs):
    from concourse.bass_utils import run_bass_kernel_spmd

    if "nc" not in _CACHE:
        _CACHE["nc"] = _build_nc()
    nc = _CACHE["nc"]

    x = np.ascontiguousarray(np.asarray(inputs["x"], dtype=np.float32))
    shared = {k: np.ascontiguousarray(np.asarray(v, dtype=np.float32))
              for k, v in inputs.items() if k != "x"}
    shared["fc_w"] = shared["fc_w"].reshape(F2, 1)
    in_maps = []
    for c in range(NCORES):
        m = dict(shared)
        m["x"] = x[c * BL:(c + 1) * BL]
        in_maps.append(m)

    res = run_bass_kernel_spmd(nc, in_maps, core_ids=list(range(NCORES)))
    y = np.concatenate([res.results[c]["y"] for c in range(NCORES)], axis=0)
    return y.reshape(B_FULL, 1, N, 1).astype(np.float32)

